# revision 1
# baseline (speedup 1.0000x reference)
"""GsLmkEncoder Trainium2 kernel.

out[n, b*68+k] = enc_b(n,k) * exp(-0.5 * wq(n,k)),   b in 0..4
  enc_0 = dz = (x_n - l_k) . rz
  enc_1 = sin(dz), enc_2 = cos(dz), enc_3 = sin(2 dz), enc_4 = cos(2 dz)
  wq = (x_n - l_k)^T cov_k (x_n - l_k)

Key reformulation: with s_n = x_n . rz and t_k = l_k . rz, dz = s_n - t_k, so
sin/cos(dz) and sin/cos(2 dz) expand by angle addition into products of
per-point trig (sin s, cos s, sin 2s, cos 2s) and per-landmark trig. wq and dz
are quadratic/linear in x. Everything therefore factors as F[n, 14] @ G[14, 6*68]
with F = [x0^2,x1^2,x2^2,x0x1,x0x2,x1x2,x0,x1,x2,1,sinS,cosS,sin2S,cos2S] and G
host-precomputed from the tiny per-landmark params. On-device per tile of 128
points: one matmul -> PSUM [128, 408] = [wq | dz | sin | cos | sin2 | cos2],
one ACT exp for w = exp(-0.5 wq), one DVE multiply (w broadcast across the 5
blocks), contiguous DMA out. ACT trig runs once in a bulk prologue over all
per-point s values, so the ACT table set switches exactly once (trig -> exp).
"""

import sys
import numpy as np

for _p in ("/opt/trn_rl_repo", "/root/.axon_site/_ro/pypackages"):
    if _p not in sys.path:
        sys.path.insert(0, _p)

import concourse.bass as bass
import concourse.bacc as bacc
import concourse.tile as tile
from concourse import mybir
from concourse.masks import make_identity
from concourse.bass_utils import run_bass_kernel_spmd

# Wire the NTFF profile hook (the agent image's antenv lacks axon_hooks);
# without it trace=True silently degrades to no profiling.
try:
    import antenv.axon_hooks  # noqa: F401
except ImportError:
    try:
        import types as _types

        sys.path.insert(0, "/root/.axon_site")
        from trn_agent_boot.trn_boot import _ntff_profile_via_ctypes

        _hook = _ntff_profile_via_ctypes("/opt/axon/libaxon_pjrt.so")
        _m = _types.ModuleType("antenv.axon_hooks")
        _m.get_axon_ntff_profile_hook = lambda: _hook
        _m.set_axon_ntff_profile_hook = lambda h: None
        sys.modules["antenv.axon_hooks"] = _m
    except Exception:
        pass

F32 = mybir.dt.float32
F32R = mybir.dt.float32r
AF = mybir.ActivationFunctionType
OP = mybir.AluOpType

N = 200000
L = 68
OUT_DIM = 5 * L  # 340
NCOLS = 6 * L    # 408: [wq | dz | s1 | c1 | s2 | c2]
K = 14           # features
NCORES = 8
NTILES = 196                 # tiles of 128 points per shard
NPAD = NTILES * 128          # 25088 per shard
TPF = 3                      # tiles per F-group (transpose granularity)
TPG = 3                      # tiles per psum/out group
KS = 32                      # feature partition stride (PE base-partition rule)
HALF_PI = float(np.pi / 2)
TWO_PI = float(np.float32(2 * np.pi))
PI_F = float(np.float32(np.pi))
INV_2PI = float(np.float32(1.0 / (2 * np.pi)))
INV_PI = float(np.float32(1.0 / np.pi))
MAGIC = 12582912.0  # 1.5 * 2**23: add+sub rounds f32 to nearest int
# clamp bounds keeping func(scale*in+bias) strictly inside [-pi, pi]
B1 = 3.141590
C1 = (-B1, B1)
C1C = (-(B1 + HALF_PI), float(np.float32(B1 - HALF_PI)))
C2 = (-B1 / 2, B1 / 2)
C2C = (-(B1 + HALF_PI) / 2, float(np.float32((B1 - HALF_PI) / 2)))


def _bcast_block(ap, nrep, block):
    """Insert a stride-0 dim of size nrep before the last dim (size block)."""
    new = ap.copy()
    pat = [list(d) for d in new.ap]
    assert pat[-1][1] == block, (pat, block)
    pat.insert(len(pat) - 1, [0, nrep])
    return bass.AP(ap.tensor, ap.offset, pat)


def build_nc(mm_f32r=True, ntiles=NTILES):
    npad = ntiles * 128
    nc = bacc.Bacc("TRN2", target_bir_lowering=False, debug=False, num_devices=NCORES)
    x_d = nc.dram_tensor("x", [npad, 3], F32, kind="ExternalInput")
    g_d = nc.dram_tensor("g", [K, NCOLS], F32, kind="ExternalInput")
    if mm_f32r:
        gt_d = nc.dram_tensor("gt", [4, 4 * L], F32R, kind="ExternalInput")
    rz_d = nc.dram_tensor("rzb", [128, 3], F32, kind="ExternalInput")
    out_d = nc.dram_tensor("out", [npad, OUT_DIM], F32, kind="ExternalOutput")

    # group sizes (tiles per F-group)
    groups = [TPF] * (ntiles // TPF)
    if ntiles % TPF:
        groups.append(ntiles % TPF)

    with tile.TileContext(nc) as tc:
        with (
            tc.tile_pool(name="const", bufs=1) as constp,
            tc.tile_pool(name="fpool", bufs=2) as fpool,
            tc.tile_pool(name="ftpool", bufs=2) as ftpool,
            tc.tile_pool(name="wpool", bufs=3) as wpool,
            tc.tile_pool(name="opool", bufs=4) as opool,
            tc.tile_pool(name="mmps", bufs=2, space="PSUM") as mmpsp,
            tc.tile_pool(name="ftps", bufs=1, space="PSUM") as ftpsp,
        ):
            # ---- persistent tiles ----
            x_sb = constp.tile([128, ntiles, 3], F32)       # grouped point layout
            s_all = constp.tile([128, ntiles], F32)
            ang = constp.tile([128, ntiles, 4], F32)
            scr = constp.tile([128, ntiles], F32)
            trig = constp.tile([128, ntiles, 4], F32)
            g_sb = constp.tile([64 + K, NCOLS], F32)
            if mm_f32r:
                gt_sb = constp.tile([64 + 4, 4 * L], F32R)
            rz_sb = constp.tile([128, 3], F32)
            ident = constp.tile([128, 128], F32)

            for _b in range(3):
                nc.sync.dma_start(g_sb[_b * KS : _b * KS + K, :], g_d[:])
                if mm_f32r:
                    nc.sync.dma_start(gt_sb[_b * KS : _b * KS + 4, :], gt_d[:])
            nc.sync.dma_start(rz_sb[:], rz_d[:])
            make_identity(nc, ident[:])
            bias_hpi = constp.tile([128, 1], F32)
            nc.gpsimd.memset(bias_hpi[:], HALF_PI)
            f_bufs = []
            f2_bufs = []
            for i in range(2):
                fb = fpool.tile([128, TPF * KS], F32, tag=f"F{i}", name=f"F{i}")
                f_bufs.append(fb)
                if mm_f32r:
                    fb2 = fpool.tile(
                        [128, TPF * KS], F32, tag=f"F2{i}", name=f"F2{i}"
                    )
                    f2_bufs.append(fb2)
            for fb in f_bufs + f2_bufs:
                nc.gpsimd.memset(fb[:], 1.0)  # col 9 stays the const-1 feature

            # x load: partition p holds points p*ntiles .. p*ntiles+ntiles-1
            # (one contiguous 2.3KB descriptor per partition)
            nc.sync.dma_start(
                x_sb[:], x_d[:].rearrange("(p m) c -> p m c", p=128)
            )

            # ---- prologue: s = x . rz, then bulk trig ----
            # absorb the x/rz DMA waits on DVE first: TensorScalarPtr
            # encodings only have one sync-wait slot
            nc.vector.tensor_tensor(
                scr[:, 0:3], x_sb[:, 0, :], rz_sb[:, 0:3], OP.mult
            )
            nc.vector.tensor_scalar(
                s_all[:], x_sb[:, :, 0], rz_sb[:, 0:1], None, OP.mult
            )
            nc.vector.scalar_tensor_tensor(
                s_all[:], x_sb[:, :, 1], rz_sb[:, 1:2], s_all[:], OP.mult, OP.add
            )
            nc.vector.scalar_tensor_tensor(
                s_all[:], x_sb[:, :, 2], rz_sb[:, 2:3], s_all[:], OP.mult, OP.add
            )
            # range-reduce the four angle families into [-pi, pi] after
            # the activation's own scale/bias is applied
            fams = [
                (INV_2PI, 0.0, -TWO_PI, C1),     # sin(s)
                (INV_2PI, 0.25, -TWO_PI, C1C),   # sin(s + pi/2)
                (INV_PI, 0.0, -PI_F, C2),        # sin(2s)
                (INV_PI, 0.25, -PI_F, C2C),      # sin(2s + pi/2)
            ]
            for ci, (inv, delta, mul, (lo, hi)) in enumerate(fams):
                # n = round(s*inv + delta) via the 1.5*2^23 magic constant;
                # delta must be added before the magic (ULP there is 1.0)
                nc.vector.tensor_scalar(
                    scr[:], s_all[:], inv, delta, OP.mult, OP.add
                )
                nc.vector.tensor_scalar(
                    scr[:], scr[:], MAGIC, MAGIC, OP.add, OP.subtract
                )
                nc.vector.scalar_tensor_tensor(
                    scr[:], scr[:], mul, s_all[:], OP.mult, OP.add
                )
                nc.vector.tensor_scalar(
                    ang[:, :, ci], scr[:], hi, lo, OP.min, OP.max
                )
            nc.scalar.activation(trig[:, :, 0], ang[:, :, 0], AF.Sin)
            nc.scalar.activation(trig[:, :, 1], ang[:, :, 1], AF.Sin, bias=bias_hpi[:])
            nc.scalar.activation(trig[:, :, 2], ang[:, :, 2], AF.Sin, scale=2.0)
            nc.scalar.activation(
                trig[:, :, 3], ang[:, :, 3], AF.Sin, bias=bias_hpi[:], scale=2.0
            )

            # ---- main loop ----
            col = 0
            gbase = 0
            for gi, tpf in enumerate(groups):
                ncols_f = tpf * KS
                f_t = f_bufs[gi % 2]
                f3 = f_t[:, 0:ncols_f].rearrange("p (t k) -> p t k", k=KS)
                xg = x_sb[:, col : col + tpf, :]
                nc.vector.tensor_tensor(f3[:, :, 0:3], xg, xg, OP.mult)
                nc.vector.tensor_tensor(
                    f3[:, :, 3:4], xg[:, :, 0:1], xg[:, :, 1:2], OP.mult
                )
                nc.vector.tensor_tensor(
                    f3[:, :, 4:5], xg[:, :, 0:1], xg[:, :, 2:3], OP.mult
                )
                nc.vector.tensor_tensor(
                    f3[:, :, 5:6], xg[:, :, 1:2], xg[:, :, 2:3], OP.mult
                )
                nc.vector.tensor_copy(f3[:, :, 6:9], xg)
                if mm_f32r:
                    f2_t = f2_bufs[gi % 2]
                    f23 = f2_t[:, 0:ncols_f].rearrange("p (t k) -> p t k", k=KS)
                    nc.vector.tensor_copy(
                        f23[:, :, 0:4], trig[:, col : col + tpf, :]
                    )
                else:
                    nc.vector.tensor_copy(
                        f3[:, :, 10:14], trig[:, col : col + tpf, :]
                    )

                ft_ps = ftpsp.tile([128, 128], F32, tag="FT")
                nc.tensor.matmul(
                    ft_ps[0:ncols_f, 0:128],
                    f_t[:, 0:ncols_f],
                    ident[:],
                    is_transpose=True,
                )
                ft_sb = ftpool.tile([128, 128], F32, tag="FTS")
                nc.scalar.copy(ft_sb[0:ncols_f, :], ft_ps[0:ncols_f, :])
                if mm_f32r:
                    ft2_ps = ftpsp.tile([128, 128], F32, tag="FT2", name="ft2_ps")
                    nc.tensor.matmul(
                        ft2_ps[0:ncols_f, 0:128],
                        f2_t[:, 0:ncols_f],
                        ident[:],
                        is_transpose=True,
                    )
                    ft2_sb = ftpool.tile([128, 128], F32R, tag="FT2S", name="ft2_sb")
                    nc.scalar.copy(ft2_sb[0:ncols_f, :], ft2_ps[0:ncols_f, :])

                out_rows = out_d[:].rearrange("(p m) c -> p (m c)", p=128)[
                    :, col * OUT_DIM : (col + tpf) * OUT_DIM
                ]

                ogs = [TPG] * (tpf // TPG)
                if tpf % TPG:
                    ogs.append(tpf % TPG)
                j0 = 0
                for tpg in ogs:
                    psum = mmpsp.tile([128, TPG, 512], F32, tag="P")
                    for jj in range(tpg):
                        j = j0 + jj
                        if mm_f32r:
                            nc.tensor.matmul(
                                psum[:, jj, 0 : 2 * L],
                                ft_sb[j * KS : j * KS + 10, 0:128],
                                g_sb[j * KS : j * KS + 10, 0 : 2 * L],
                                start=True,
                                stop=True,
                            )
                            nc.tensor.matmul(
                                psum[:, jj, 2 * L : NCOLS],
                                ft2_sb[j * KS : j * KS + 4, 0:128],
                                gt_sb[j * KS : j * KS + 4, :],
                                start=True,
                                stop=True,
                            )
                        else:
                            nc.tensor.matmul(
                                psum[:, jj, 0:NCOLS],
                                ft_sb[j * KS : j * KS + K, 0:128],
                                g_sb[j * KS : j * KS + K, :],
                                start=True,
                                stop=True,
                            )
                    w_t = wpool.tile([128, TPG, L], F32, tag="W")
                    nc.scalar.activation(
                        w_t[:, 0:tpg, :], psum[:, 0:tpg, 0:L], AF.Exp, scale=-0.5
                    )
                    o_t = opool.tile([128, TPG * OUT_DIM], F32, tag="O")
                    enc = psum[:, 0:tpg, L:NCOLS].rearrange(
                        "p t (b l) -> p t b l", l=L
                    )
                    o4 = o_t[:, 0 : tpg * OUT_DIM].rearrange(
                        "p (t b l) -> p t b l", b=5, l=L
                    )
                    wb = _bcast_block(w_t[:, 0:tpg, :], 5, L)
                    nc.vector.tensor_tensor(o4, enc, wb, OP.mult)
                    nc.sync.dma_start(
                        out_rows[:, j0 * OUT_DIM : (j0 + tpg) * OUT_DIM],
                        o_t[:, 0 : tpg * OUT_DIM],
                    )
                    j0 += tpg
                col += tpf
                gbase += 128 * tpf
    nc.compile()
    return nc


def host_params(l, r, scaling, rotation):
    """G [14, 408] float32 + rz broadcast, mirroring reference math."""
    l = l.astype(np.float64)
    r = r.astype(np.float64)
    scaling = scaling.astype(np.float64)
    rotation = rotation.astype(np.float64)

    rz = r[:3, 2]
    qn = rotation / np.maximum(
        np.linalg.norm(rotation, axis=1, keepdims=True), 1e-12
    )
    w, x, y, z = qn[:, 0], qn[:, 1], qn[:, 2], qn[:, 3]
    R = np.empty((L, 3, 3), np.float64)
    R[:, 0, 0] = 1 - 2 * (y * y + z * z)
    R[:, 0, 1] = 2 * (x * y - w * z)
    R[:, 0, 2] = 2 * (x * z + w * y)
    R[:, 1, 0] = 2 * (x * y + w * z)
    R[:, 1, 1] = 1 - 2 * (x * x + z * z)
    R[:, 1, 2] = 2 * (y * z - w * x)
    R[:, 2, 0] = 2 * (x * z - w * y)
    R[:, 2, 1] = 2 * (y * z + w * x)
    R[:, 2, 2] = 1 - 2 * (x * x + y * y)
    M = R / scaling[:, None, :]
    cov = np.einsum("lij,lkj->lik", M, M)       # [L,3,3]

    b = np.einsum("lij,lj->li", cov, l)         # cov_k @ l_k
    c = np.einsum("li,li->l", l, b)             # l^T cov l
    t = l @ rz
    G = np.zeros((K, NCOLS), np.float64)
    # wq block
    G[0, 0:L] = cov[:, 0, 0]
    G[1, 0:L] = cov[:, 1, 1]
    G[2, 0:L] = cov[:, 2, 2]
    G[3, 0:L] = 2 * cov[:, 0, 1]
    G[4, 0:L] = 2 * cov[:, 0, 2]
    G[5, 0:L] = 2 * cov[:, 1, 2]
    G[6:9, 0:L] = -2 * b.T
    G[9, 0:L] = c
    # dz block
    G[6:9, L : 2 * L] = rz[:, None] * np.ones((1, L))
    G[9, L : 2 * L] = -t
    # trig blocks: sin(s-t) = sinS cosT - cosS sinT ; cos(s-t) = cosS cosT + sinS sinT
    c1, s1 = np.cos(t), np.sin(t)
    c2, s2 = np.cos(2 * t), np.sin(2 * t)
    G[10, 2 * L : 3 * L] = c1
    G[11, 2 * L : 3 * L] = -s1
    G[10, 3 * L : 4 * L] = s1
    G[11, 3 * L : 4 * L] = c1
    G[12, 4 * L : 5 * L] = c2
    G[13, 4 * L : 5 * L] = -s2
    G[12, 5 * L : 6 * L] = s2
    G[13, 5 * L : 6 * L] = c2
    return G.astype(np.float32), np.broadcast_to(
        rz.astype(np.float32), (128, 3)
    ).copy()


_NC_CACHE = {}


def _get_nc(mm_f32r=True):
    key = bool(mm_f32r)
    if key not in _NC_CACHE:
        _NC_CACHE[key] = build_nc(mm_f32r=key)
    return _NC_CACHE[key]


def run(inputs, mm_f32r=True, trace=False):
    x = inputs["x"]
    G, rzb = host_params(
        inputs["l"], inputs["r"], inputs["scaling"], inputs["rotation"]
    )
    xpad = np.zeros((NCORES * NPAD, 3), np.float32)
    xpad[:N] = x
    shards = xpad.reshape(NCORES, NPAD, 3)
    in_maps = []
    for i in range(NCORES):
        m = {"x": np.ascontiguousarray(shards[i]), "g": G, "rzb": rzb}
        if mm_f32r:
            m["gt"] = np.ascontiguousarray(G[10:14, 2 * L :])
        in_maps.append(m)
    nc = _get_nc(mm_f32r)
    res = run_bass_kernel_spmd(nc, in_maps, list(range(NCORES)), trace=trace)
    out = np.concatenate([r["out"] for r in res.results], axis=0)[:N]
    return out, res


def kernel(**inputs):
    out, _ = run(inputs)
    return out

